# revision 14
# baseline (speedup 1.0000x reference)
"""Multi-head causal attention (B=4, T=2048, D=1024, H=16) on 8 TRN2 cores.

Tensor-parallel over heads: core c computes heads {2c, 2c+1}. Each core:
  - Q', K', V' feature-major ([feat, tok]) via 512-wide PE matmuls,
  - V' -> token-major V via PE transposes, stored as [v_h0 | ones(64) | v_h1]
    so each head's PV lhsT slice carries 64 ones-columns that replicate the
    softmax denominator across 64 PSUM partitions,
  - S^T = K'^T Q' tiles [128 k x 512 q], h0/h1 issued back-to-back so the
    64-deep score matmuls run concurrently on PE row groups 0-63/64-127,
  - exp (no max subtraction: |S|/32 <= ~2), multiplicative 0/1 causal mask,
    diagonal blocks narrowed to their live query range,
  - PV accumulates [y_h | denom replicas] per psum bank; normalization is one
    in-place DVE reciprocal_approx_fast + one partition-offset tensor_tensor,
  - w_proj row-slice partial matmuls emitted per query chunk with lag-1 so the
    projection/output DMA pipelines under the next chunk's attention,
  - partial projections written bf16; host sums the 8 cores and transposes.
"""

import sys

for _p in ("/opt/trn_rl_repo",):
    if _p not in sys.path:
        sys.path.append(_p)

import numpy as np
import ml_dtypes

B, T, D = 4, 2048, 1024
H = 16
HD = D // H
NORM = float(np.sqrt(D))
N_CORES = 8
HEADS_PER_CORE = H // N_CORES          # 2
FPC = HEADS_PER_CORE * HD              # 128 features per core
QC = 512                               # query chunk
NQC = T // QC                          # 4
KB = 128                               # key block
DKC = D // 128                         # 8 contraction chunks over D

_BF16 = ml_dtypes.bfloat16

_cache = {}


def _build():
    import concourse.bacc as bacc
    import concourse.mybir as mybir
    from concourse.tile import TileContext
    from concourse.alu_op_type import AluOpType
    from concourse.masks import make_identity

    f32 = mybir.dt.float32
    bf16 = mybir.dt.bfloat16
    EXP = mybir.ActivationFunctionType.Exp

    nc = bacc.Bacc("TRN2", target_bir_lowering=False, debug=False,
                   num_devices=N_CORES)

    xt = nc.dram_tensor("xt", [B, D, T], bf16, kind="ExternalInput").ap()
    w3 = nc.dram_tensor("w3", [D, 3 * FPC], bf16, kind="ExternalInput").ap()
    wp = nc.dram_tensor("wp", [FPC, D], bf16, kind="ExternalInput").ap()
    masks = nc.dram_tensor("masks", [4, KB, QC], bf16, kind="ExternalInput").ap()
    out = nc.dram_tensor("out", [B, D, T], bf16, kind="ExternalOutput").ap()

    with TileContext(nc) as tc:
        with (
            tc.tile_pool(name="const", bufs=1) as cpool,
            tc.tile_pool(name="xp", bufs=10) as xpool,
            tc.tile_pool(name="qk", bufs=2) as qkpool,
            tc.tile_pool(name="vaug", bufs=24) as vpool,
            tc.tile_pool(name="pt", bufs=30) as ptpool,
            tc.tile_pool(name="y", bufs=4) as ypool,
            tc.tile_pool(name="ot", bufs=6) as otpool,
            tc.tile_pool(name="rec", bufs=2) as recpool,
            tc.tile_pool(name="psA", bufs=2, space="PSUM") as psA,
            tc.tile_pool(name="psY", bufs=2, space="PSUM") as psY,
            tc.tile_pool(name="psO", bufs=2, space="PSUM") as psO,
        ):
            # ---- constants ----
            w3_t = []
            for kc in range(DKC):
                t = cpool.tile([128, 3 * FPC], bf16, tag=f"w3{kc}")
                nc.sync.dma_start(t[:], w3[kc * 128:(kc + 1) * 128, :])
                w3_t.append(t)
            wp_t = cpool.tile([FPC, D], bf16, tag="wp")
            nc.sync.dma_start(wp_t[:], wp[:])
            mask_t = []
            for p in range(4):
                t = cpool.tile([KB, QC], bf16, tag=f"mask{p}")
                nc.sync.dma_start(t[:], masks[p])
                mask_t.append(t)
            ident = cpool.tile([128, 128], bf16, tag="ident")
            make_identity(nc, ident[:])

            # PE warmup during the initial x DMA: keeps the HAM clock-gate
            # busy so real matmuls start at 2.4 GHz.
            psw = psO.tile([128, QC], f32, tag="pso")
            for _ in range(90):
                nc.tensor.matmul(psw[:, 0:128], lhsT=ident[:], rhs=ident[:],
                                 start=True, stop=True)

            def emit_qkv_ft(b, ft, xp_t, dst):
                # one of Q'/K'/V' feature-major [128, T]; same-weight matmuls
                # issued back-to-back so LDWEIGHTS amortizes over 2 chunks.
                with nc.named_scope("qkv"):
                    for np2 in range(NQC // 2):
                        ps = psA.tile([128, 2 * QC], f32, tag="ps")
                        for kc in range(DKC):
                            for half in range(2):
                                ntk = 2 * np2 + half
                                nc.tensor.matmul(
                                    ps[:, QC * half:QC * (half + 1)],
                                    lhsT=w3_t[kc][:, 128 * ft:128 * (ft + 1)],
                                    rhs=xp_t[kc][:, QC * ntk:QC * (ntk + 1)],
                                    start=(kc == 0), stop=(kc == DKC - 1),
                                )
                        nc.scalar.copy(
                            dst[:, 2 * QC * np2:2 * QC * (np2 + 1)], ps[:])

            def emit_vtrans(b, vp):
                # V' -> token-major V, layout [ones|v_h0|ones|v_h1] so each
                # head's 128-wide PV lhsT slice is [ones(64) | v_h(64)] and
                # the denominator replicas land on PSUM partitions 0-63
                # (the HW custom-DVE reciprocal reads partition base 0).
                vaug_t = []
                with nc.named_scope("vtrans"):
                    for tk in range(T // 128):
                        ps = psY.tile([128, FPC], bf16, tag="psy")
                        nc.tensor.transpose(
                            ps[:], vp[:, 128 * tk:128 * (tk + 1)], ident[:]
                        )
                        va = vpool.tile([128, 4 * HD], bf16, tag="vaug")
                        nc.vector.tensor_copy(va[:, HD:2 * HD], ps[:, 0:HD])
                        nc.vector.tensor_copy(va[:, 3 * HD:4 * HD], ps[:, HD:2 * HD])
                        nc.gpsimd.memset(va[:, 0:HD], 1.0)
                        nc.gpsimd.memset(va[:, 2 * HD:3 * HD], 1.0)
                        vaug_t.append(va)
                return vaug_t

            def score_pair(qc, h, kb2, pts, qp, kp):
                # scores for key blocks (kbA, kbB) x one 512-query chunk in a
                # [128, 1024] f32 psum tile; one exp covers the whole live
                # range (the dead gap between halves is never read by PV).
                kbA, kbB = 2 * kb2, 2 * kb2 + 1
                j0A = max(0, KB * (kbA - qc * 4))
                j0B = max(0, KB * (kbB - qc * 4))
                pss = psA.tile([128, 2 * QC], f32, tag="ps")
                for off, kb, j0 in ((0, kbA, j0A), (QC, kbB, j0B)):
                    nc.tensor.matmul(
                        pss[:, off + j0:off + QC],
                        lhsT=kp[HD * h:HD * (h + 1), KB * kb:KB * (kb + 1)],
                        rhs=qp[HD * h:HD * (h + 1), QC * qc + j0:QC * (qc + 1)],
                        start=True, stop=True,
                    )
                pt = ptpool.tile([KB, 2 * QC], bf16, tag="pt")
                nc.scalar.activation(pt[:, j0A:2 * QC], pss[:, j0A:2 * QC],
                                     EXP, scale=1.0 / NORM)
                for off, kb, j0 in ((0, kbA, j0A), (QC, kbB, j0B)):
                    p = kb - qc * 4
                    if p >= 0:
                        nc.vector.tensor_tensor(
                            pt[:, off + j0:off + QC],
                            pt[:, off + j0:off + QC],
                            mask_t[p][:, j0:QC],
                            op=AluOpType.mult,
                        )
                    pts[qc, h, kb] = (pt, off, j0)

            def emit_scores(qc, pts, qp, kp):
                nkb = (qc + 1) * (QC // KB)
                with nc.named_scope("score"):
                    for kb2 in range(nkb // 2):
                        score_pair(qc, 0, kb2, pts, qp, kp)
                        score_pair(qc, 1, kb2, pts, qp, kp)

            def emit_pv(qc, pts, vaug_t, ys):
                # PV with denominator replicas; normalize in-place on DVE.
                nkb = (qc + 1) * (QC // KB)
                kb_order = [kb for kb in range(nkb) if kb < qc * 4] + \
                           [kb for kb in range(nkb) if kb >= qc * 4]
                y = ypool.tile([FPC, QC], bf16, tag="y")
                with nc.named_scope("pv"):
                    for h in range(HEADS_PER_CORE):
                        psy = psY.tile([128, QC], f32, tag="psy")
                        for i, kb in enumerate(kb_order):
                            pt, off, j0 = pts[qc, h, kb]
                            lo = 2 * HD * h
                            nc.tensor.matmul(
                                psy[:, j0:QC],
                                lhsT=vaug_t[kb][:, lo:lo + 128],
                                rhs=pt[:, off + j0:off + QC],
                                start=(i == 0), stop=(i == nkb - 1),
                            )
                        # rows 0-63 = denom replicas, 64-127 = y for both
                        # heads. DVE reads at most one PSUM operand per
                        # instruction, so the reciprocal lands in SBUF.
                        rec = recpool.tile([64, QC], f32, tag="rec")
                        nc.vector.reciprocal_approx_fast(
                            rec[:], psy[0:64, :])
                        nc.vector.tensor_tensor(
                            y[HD * h:HD * (h + 1), :],
                            psy[64:128, :],
                            rec[:],
                            op=AluOpType.mult,
                        )
                ys[qc] = y

            def emit_proj(b, qc, ys):
                with nc.named_scope("proj"):
                    y = ys[qc]
                    for mt in range(D // 128):
                        pso = psO.tile([128, QC], f32, tag="pso")
                        nc.tensor.matmul(
                            pso[:],
                            lhsT=wp_t[:, 128 * mt:128 * (mt + 1)],
                            rhs=y[:],
                            start=True, stop=True,
                        )
                        ot = otpool.tile([128, QC], bf16, tag="ot")
                        nc.vector.tensor_copy(ot[:], pso[:])
                        nc.sync.dma_start(
                            out[b, 128 * mt:128 * (mt + 1), QC * qc:QC * (qc + 1)],
                            ot[:],
                        )

            def emit_xload(b):
                xp_t = []
                for kc in range(DKC):
                    t = xpool.tile([128, T], bf16, tag="xp")
                    nc.sync.dma_start(t[:], xt[b, kc * 128:(kc + 1) * 128, :])
                    xp_t.append(t)
                return xp_t

            # Software-pipelined emission: QKV/vtrans of batch b+1 are
            # interleaved into batch b's attention so the static PE stream
            # always has dependency-free matmuls to run while ACT works
            # through the exps. The last batch runs its query chunks in
            # reverse so the drain tail ends on the shortest chunk.
            st = [dict() for _ in range(B)]
            st[0]['xp'] = emit_xload(0)
            st[0]['qkv'] = tuple(
                qkpool.tile([128, T], bf16, tag=t, name=f"{t}0")
                for t in ("qp", "kp", "vp"))
            for ft in range(3):
                emit_qkv_ft(0, ft, st[0]['xp'], st[0]['qkv'][ft])
            st[0]['vaug'] = emit_vtrans(0, st[0]['qkv'][2])

            for b in range(B):
                s = st[b]
                s.setdefault('pts', {})
                s.setdefault('ys', {})
                s.setdefault('exp', [])
                qp, kp = s['qkv'][0], s['qkv'][1]
                nxt = st[b + 1] if b + 1 < B else None
                if nxt is not None:
                    nxt['xp'] = emit_xload(b + 1)
                    nxt['qkv'] = tuple(
                        qkpool.tile([128, T], bf16, tag=t, name=f"{t}{b + 1}")
                        for t in ("qp", "kp", "vp"))
                def qkv1(ft):
                    if nxt is not None:
                        emit_qkv_ft(b + 1, ft, nxt['xp'], nxt['qkv'][ft])

                qcs = list(range(NQC)) if b < B - 1 else [3, 2, 1, 0]
                emit_scores(qcs[0], s['pts'], qp, kp)
                emit_scores(qcs[1], s['pts'], qp, kp)
                emit_pv(qcs[0], s['pts'], s['vaug'], s['ys'])
                qkv1(0)
                emit_scores(qcs[2], s['pts'], qp, kp)
                emit_pv(qcs[1], s['pts'], s['vaug'], s['ys'])
                emit_proj(b, qcs[0], s['ys'])
                qkv1(1)
                emit_scores(qcs[3], s['pts'], qp, kp)
                emit_pv(qcs[2], s['pts'], s['vaug'], s['ys'])
                emit_proj(b, qcs[1], s['ys'])
                qkv1(2)
                emit_pv(qcs[3], s['pts'], s['vaug'], s['ys'])
                emit_proj(b, qcs[2], s['ys'])
                emit_proj(b, qcs[3], s['ys'])
                if nxt is not None:
                    nxt['vaug'] = emit_vtrans(b + 1, nxt['qkv'][2])

    nc.compile()
    return nc


def _get_nc():
    if "nc" not in _cache:
        _cache["nc"] = _build()
    return _cache["nc"]


def _make_masks():
    i = np.arange(KB)[:, None]
    j = np.arange(QC)[None, :]
    m = np.zeros((4, KB, QC), dtype=np.float32)
    for p in range(4):
        m[p] = (j >= (KB * p + i)).astype(np.float32)
    return m.astype(_BF16)


def shard_inputs(x, w_qkv, w_proj):
    xt = np.ascontiguousarray(np.asarray(x, dtype=np.float32).transpose(0, 2, 1))
    xt = xt.astype(_BF16)
    w_qkv = np.asarray(w_qkv, dtype=np.float32)
    w_proj = np.asarray(w_proj, dtype=np.float32)
    masks = _make_masks()
    in_maps = []
    for c in range(N_CORES):
        qcols = slice(FPC * c, FPC * (c + 1))
        kcols = slice(D + FPC * c, D + FPC * (c + 1))
        vcols = slice(2 * D + FPC * c, 2 * D + FPC * (c + 1))
        w3_c = np.concatenate(
            [w_qkv[:, qcols], w_qkv[:, kcols], w_qkv[:, vcols]], axis=1)
        in_maps.append({
            "xt": xt,
            "w3": np.ascontiguousarray(w3_c).astype(_BF16),
            "wp": np.ascontiguousarray(w_proj[FPC * c:FPC * (c + 1), :]).astype(_BF16),
            "masks": masks,
        })
    return in_maps


def unshard(results):
    total = results[0]["out"].astype(np.float32)
    for r in results[1:]:
        total += r["out"].astype(np.float32)
    return np.ascontiguousarray(total.transpose(0, 2, 1))


def run(inputs, trace=False, **kw):
    from concourse.bass_utils import run_bass_kernel_spmd

    nc = _get_nc()
    in_maps = shard_inputs(inputs["x"], inputs["w_qkv"], inputs["w_proj"])
    res = run_bass_kernel_spmd(nc, in_maps, core_ids=list(range(N_CORES)),
                               trace=trace, **kw)
    return unshard(res.results), res


def kernel(**inputs):
    out, _ = run(inputs, trace=False)
    return out
